# revision 33
# baseline (speedup 1.0000x reference)
"""Trainium2 Bass kernel for nn_BilinearSparseRouting (FC capsule routing layer).

Math (after constant-folding the softmax-over-a-constant, which is exactly 1/32):
    cp2[b,j]   = (pose[b,j] as 4x4) @ wc[j]            # (4,4) each
    S[b]       = (1/32) * sum_j cp2[b,j]               # (4,4)
    out[b,o]   = S[b] @ wn[o]                          # (4,4), o = 0..31
    output shape (256, 1, 1, 32, 16)

Device strategy (data-parallel over batch, 32 batches per core):
  Stage 1 is a 16384-term contraction per (b, r):
      T[(b,r), c] = sum_{(j,k)} pose[b, j, 4r+k] * wc[j, k, c]

  fp32 matmuls on the PE run as 2 half-speed passes (4 cyc/col); instead we
  split both operands into fp16 hi+lo pairs (x = x_hi + x_lo captures 22
  mantissa bits; fp16*fp16 products are exact in the fp32 PSUM accumulate).
  Packing rhs = [x_hi | x_lo] (256 cols) and lhsT = [w_hi | w_lo] (8 cols)
  computes all four cross products in ONE fp16 matmul per 128-row chunk:

      psum1[c + 4*hw, (b,r) + 128*hx] += w_hw_chunk.T @ x_hx_chunk

  at 1 cyc/col -- 4x faster than fp32 with ~fp32 accuracy (the lo*lo term
  is included, cutting the residual to ~2^-22).

  Stage 2 folds the quadrant collapse + the exact 1/32 scale into a single
  contraction against wn/32 replicated over the hw axis:

      out[(b,r),(o,c)] = sum_{k2,hw} s8[(k2,hw), (b,r)+128*hx] * wn8[(k2,hw),(o,c)]

  accumulated over hx in 2 tiny fp32 matmuls.

  The 8 MiB/core x stream is laid out on the host as per-group dense
  contiguous DRAM regions and streamed on the sync HWDGE ring with all
  destination tiles SBUF-resident, so the DMAs queue back-to-back at the
  practical HBM rate (~295 GB/s/core measured with all 8 cores streaming).
"""

import os
import sys

for _p in ("/opt/trn_rl_repo", "/root/.axon_site/_ro/trn_rl_repo"):
    if _p not in sys.path:
        sys.path.insert(0, _p)

# The kernel executes through the axon PJRT backend; a leftover cpu pin from a
# reference-running harness would hide the NeuronCores if jax has not
# initialized its backend yet.
os.environ.pop("JAX_PLATFORMS", None)

from contextlib import ExitStack  # noqa: E402

import numpy as np  # noqa: E402

import concourse.bacc as bacc  # noqa: E402
import concourse.mybir as mybir  # noqa: E402
import concourse.tile as tile  # noqa: E402
from concourse.bass_utils import run_bass_kernel_spmd  # noqa: E402

B = 256
N_IN = 4096
N_OUT = 32
MPD = 4
POSE_DIM = 16
N_CORES = 8
B_SH = B // N_CORES            # 32 batches per core
JK = N_IN * MPD                # 16384 contraction terms
NCHUNK = JK // 128             # 128 PE matmuls
XCOLS = NCHUNK * 256           # fp16 hi|lo packed columns of x

F32 = mybir.dt.float32
F16 = mybir.dt.float16

# Built once, reused across kernel() calls.
_CACHE = {}

# test.py hooks: set TRACE=True before calling kernel() to profile; the
# BassKernelResults of the last run lands in LAST_RESULT.
TRACE = False
TRACE_KWARGS = {}
LAST_RESULT = None


def _build_program():
    nc = bacc.Bacc("TRN2", target_bir_lowering=False, debug=False,
                   num_devices=N_CORES)
    wn = nc.dram_tensor("wn", [8, N_OUT * MPD], F32, kind="ExternalInput").ap()
    y = nc.dram_tensor("y", [128, 128], F32, kind="ExternalOutput").ap()

    # Group boundaries in chunks: small first group so the matmul stream
    # starts early, then a geometrically tapering tail.  A group's matmuls
    # can only start once its whole DMA lands (sem granularity), and the PE
    # consumes ~0.11 us/chunk vs ~0.18 us/chunk delivery -- so each trailing
    # group at <= ~1.4x the size of the next keeps the PE finishing a group
    # right as the next lands, cutting the post-stream PE trail from ~3 us
    # (one 30-chunk group) to ~0.5 us.
    bounds = [0, 4, 32, 60, 84, 101, 113, 121, 125, 128]
    assert bounds[-1] == NCHUNK

    # One DRAM tensor per stream group: each group is a dense contiguous
    # region (partition stride = the group's row length), giving the HBM
    # reads a compact footprint instead of 64 KiB-strided rows.  Group 0
    # carries the stage-1 weights prepended to its columns, so one DMA
    # delivers everything the first matmuls need.
    W8 = NCHUNK * 8
    xg = [
        nc.dram_tensor(
            f"x{g}",
            [128, (bounds[g + 1] - bounds[g]) * 256 + (W8 if g == 0 else 0)],
            F16, kind="ExternalInput").ap()
        for g in range(len(bounds) - 1)
    ]

    with tile.TileContext(nc) as tc, ExitStack() as ctx:
        wpool = ctx.enter_context(tc.tile_pool(name="wpool", bufs=1))
        # All x groups stay resident (8 MiB) so every stream DMA can be
        # issued up front; the sync HWDGE ring then drains back-to-back at
        # the HBM rate with no buffer-release gating.
        xpool = ctx.enter_context(tc.tile_pool(name="xpool", bufs=1))
        opool = ctx.enter_context(tc.tile_pool(name="opool", bufs=1))
        ppool = ctx.enter_context(tc.tile_pool(name="ppool", bufs=1, space="PSUM"))

        wn_sb = wpool.tile([8, N_OUT * MPD], F32, tag="wn")
        nc.scalar.dma_start(wn_sb[:], wn[:])

        # Stage 1: one fp16 matmul per 128-row chunk covers all 4 hi/lo
        # cross products; accumulate everything into one (8, 256) psum.
        psum1 = ppool.tile([8, 256], F32, tag="t")
        xts = []
        n_groups = len(bounds) - 1
        for g in range(n_groups):
            c0, c1 = bounds[g], bounds[g + 1]
            ncols = (c1 - c0) * 256 + (W8 if g == 0 else 0)
            xt = xpool.tile([128, ncols], F16, tag=f"x{g}")
            nc.sync.dma_start(xt[:], xg[g][:])
            xts.append(xt)
        w_sb = xts[0][:, 0:W8]
        for g in range(n_groups):
            c0, c1 = bounds[g], bounds[g + 1]
            xt = xts[g]
            off = W8 if g == 0 else 0
            for jj in range(c1 - c0):
                c = c0 + jj
                nc.tensor.matmul(
                    psum1[:],
                    lhsT=w_sb[:, c * 8:(c + 1) * 8],
                    rhs=xt[:, off + jj * 256:off + (jj + 1) * 256],
                    start=(c == 0),
                    stop=(c == NCHUNK - 1),
                )

        # Evacuate psum1 in halves so the first stage-2 matmul overlaps the
        # second half's copy (slice-level deps, same mechanism as w_sb).
        s8 = opool.tile([8, 256], F32, tag="s8")
        nc.vector.tensor_copy(s8[:, 0:128], psum1[:, 0:128])
        nc.vector.tensor_copy(s8[:, 128:256], psum1[:, 128:256])

        # Stage 2: contract over (k2, hw) against wn/32 (host-prescaled,
        # exact power-of-2), accumulating the two hx halves.
        psum2 = ppool.tile([128, 128], F32, tag="out")
        nc.tensor.matmul(psum2[:], lhsT=s8[:, 0:128], rhs=wn_sb[:],
                         start=True, stop=False)
        nc.tensor.matmul(psum2[:], lhsT=s8[:, 128:256], rhs=wn_sb[:],
                         start=False, stop=True)
        out_sb = opool.tile([128, 128], F32, tag="y")
        nc.vector.tensor_copy(out_sb[:], psum2[:])
        nc.sync.dma_start(y[:], out_sb[:])

    nc.compile()
    _CACHE["bounds"] = bounds
    return nc


def _split_f16(a: np.ndarray):
    hi = a.astype(np.float16)
    lo = (a - hi.astype(np.float32)).astype(np.float16)
    return hi, lo


def _prep_x(current_pose: np.ndarray) -> np.ndarray:
    """(256, 4096, 16) -> (8 cores, 128 partitions, NCHUNK*256 fp16 cols).

    Per core the stage-1 contraction matrix has row index (j*4 + k) and
    column (b*4 + r) with element pose[b, j, 4r+k].  Chunk Jc's 128x128
    tile lands in packed columns [Jc*256, Jc*256+128) as fp16 hi and
    [Jc*256+128, (Jc+1)*256) as fp16 lo.
    """
    a = current_pose.reshape(N_CORES, B_SH, N_IN, MPD, MPD)   # m b j r k
    t = a.transpose(0, 2, 4, 1, 3)                            # m j k b r
    c = t.reshape(N_CORES, NCHUNK, 128, 128)                  # m Jc p col
    c = np.ascontiguousarray(c.transpose(0, 2, 1, 3))         # m p Jc col
    hi, lo = _split_f16(c)
    packed = np.stack([hi, lo], axis=3)                       # m p Jc {hi,lo} col
    return np.ascontiguousarray(packed.reshape(N_CORES, 128, XCOLS))


def kernel(current_pose, w_current, w_next, h_out=1, w_out=1):
    global LAST_RESULT
    current_pose = np.asarray(current_pose, dtype=np.float32)
    w_current = np.asarray(w_current, dtype=np.float32)
    w_next = np.asarray(w_next, dtype=np.float32)

    if not TRACE:
        # bass_utils would honor a stray BASS_TRACE env var and then crash on
        # this image's missing NTFF hook module.
        os.environ.pop("BASS_TRACE", None)

    if "nc" not in _CACHE:
        _CACHE["nc"] = _build_program()
    nc = _CACHE["nc"]
    bounds = _CACHE["bounds"]

    xs = _prep_x(current_pose)

    # wc[j,k,c] flattened over rows (j,k); chunk Jc's (128, 4) block packed
    # into SBUF-image columns [Jc*8, Jc*8+4) as fp16 hi, [Jc*8+4, +8) as lo.
    wc_flat = w_current.reshape(JK, MPD)
    whi, wlo = _split_f16(wc_flat)
    w_img = np.concatenate(
        [whi.reshape(NCHUNK, 128, MPD), wlo.reshape(NCHUNK, 128, MPD)], axis=2)
    w_img = np.ascontiguousarray(
        w_img.transpose(1, 0, 2).reshape(128, NCHUNK * 8))

    # wn arranged (k2, (o,c)), pre-scaled by the exact 1/32 softmax constant
    # and replicated over the w-hi/lo axis for the stage-2 collapse.
    wn_t = (w_next.transpose(1, 0, 2).reshape(MPD, N_OUT * MPD)
            * np.float32(1.0 / N_OUT))
    wn8 = np.ascontiguousarray(np.concatenate([wn_t, wn_t], axis=0))

    in_maps = [
        {"wn": wn8,
         "x0": np.ascontiguousarray(np.concatenate(
             [w_img, xs[m][:, bounds[0] * 256:bounds[1] * 256]], axis=1)),
         **{f"x{g}": np.ascontiguousarray(
                xs[m][:, bounds[g] * 256:bounds[g + 1] * 256])
            for g in range(1, len(bounds) - 1)}}
        for m in range(N_CORES)
    ]
    res = run_bass_kernel_spmd(nc, in_maps, list(range(N_CORES)), trace=TRACE,
                               **TRACE_KWARGS)
    LAST_RESULT = res

    out = np.empty((B, 1, 1, N_OUT, POSE_DIM), dtype=np.float32)
    for m in range(N_CORES):
        ym = res.results[m]["y"]                      # (128=(b,r), 128=(o,c))
        out[m * B_SH:(m + 1) * B_SH, 0, 0] = (
            ym.reshape(B_SH, MPD, N_OUT, MPD)
            .transpose(0, 2, 1, 3).reshape(B_SH, N_OUT, POSE_DIM))
    return out


# revision 34
# speedup vs baseline: 1.0501x; 1.0501x over previous
"""Trainium2 Bass kernel for nn_BilinearSparseRouting (FC capsule routing layer).

Math (after constant-folding the softmax-over-a-constant, which is exactly 1/32):
    cp2[b,j]   = (pose[b,j] as 4x4) @ wc[j]            # (4,4) each
    S[b]       = (1/32) * sum_j cp2[b,j]               # (4,4)
    out[b,o]   = S[b] @ wn[o]                          # (4,4), o = 0..31
    output shape (256, 1, 1, 32, 16)

Device strategy (data-parallel over batch, 32 batches per core):
  Stage 1 is a 16384-term contraction per (b, r):
      T[(b,r), c] = sum_{(j,k)} pose[b, j, 4r+k] * wc[j, k, c]

  fp32 matmuls on the PE run as 2 half-speed passes (4 cyc/col); instead we
  split both operands into fp16 hi+lo pairs (x = x_hi + x_lo captures 22
  mantissa bits; fp16*fp16 products are exact in the fp32 PSUM accumulate).
  Packing rhs = [x_hi | x_lo] (256 cols) and lhsT = [w_hi | w_lo] (8 cols)
  computes all four cross products in ONE fp16 matmul per 128-row chunk:

      psum1[c + 4*hw, (b,r) + 128*hx] += w_hw_chunk.T @ x_hx_chunk

  at 1 cyc/col -- 4x faster than fp32 with ~fp32 accuracy (the lo*lo term
  is included, cutting the residual to ~2^-22).

  Stage 2 folds the quadrant collapse + the exact 1/32 scale into a single
  contraction against wn/32 replicated over the hw axis:

      out[(b,r),(o,c)] = sum_{k2,hw} s8[(k2,hw), (b,r)+128*hx] * wn8[(k2,hw),(o,c)]

  accumulated over hx in 2 tiny fp32 matmuls.

  The 8 MiB/core x stream is laid out on the host as per-group dense
  contiguous DRAM regions and streamed on the sync HWDGE ring with all
  destination tiles SBUF-resident, so the DMAs queue back-to-back at the
  practical HBM rate (~295 GB/s/core measured with all 8 cores streaming).
"""

import os
import sys

for _p in ("/opt/trn_rl_repo", "/root/.axon_site/_ro/trn_rl_repo"):
    if _p not in sys.path:
        sys.path.insert(0, _p)

# The kernel executes through the axon PJRT backend; a leftover cpu pin from a
# reference-running harness would hide the NeuronCores if jax has not
# initialized its backend yet.
os.environ.pop("JAX_PLATFORMS", None)

from contextlib import ExitStack  # noqa: E402

import numpy as np  # noqa: E402

import concourse.bacc as bacc  # noqa: E402
import concourse.mybir as mybir  # noqa: E402
import concourse.tile as tile  # noqa: E402
from concourse.bass_utils import run_bass_kernel_spmd  # noqa: E402

B = 256
N_IN = 4096
N_OUT = 32
MPD = 4
POSE_DIM = 16
N_CORES = 8
B_SH = B // N_CORES            # 32 batches per core
JK = N_IN * MPD                # 16384 contraction terms
NCHUNK = JK // 128             # 128 PE matmuls
XCOLS = NCHUNK * 256           # fp16 hi|lo packed columns of x

F32 = mybir.dt.float32
F16 = mybir.dt.float16

# Built once, reused across kernel() calls.
_CACHE = {}

# test.py hooks: set TRACE=True before calling kernel() to profile; the
# BassKernelResults of the last run lands in LAST_RESULT.
TRACE = False
TRACE_KWARGS = {}
LAST_RESULT = None


def _build_program():
    nc = bacc.Bacc("TRN2", target_bir_lowering=False, debug=False,
                   num_devices=N_CORES)
    wn = nc.dram_tensor("wn", [8, N_OUT * MPD], F32, kind="ExternalInput").ap()
    y = nc.dram_tensor("y", [128, 128], F32, kind="ExternalOutput").ap()

    # Group boundaries in chunks: small first group so the matmul stream
    # starts early, then a geometrically tapering tail.  A group's matmuls
    # can only start once its whole DMA lands (sem granularity), and the PE
    # consumes ~0.11 us/chunk vs ~0.18 us/chunk delivery -- so each trailing
    # group at <= ~1.4x the size of the next keeps the PE finishing a group
    # right as the next lands, cutting the post-stream PE trail from ~3 us
    # (one 30-chunk group) to ~0.5 us.
    bounds = [0, 4, 32, 60, 84, 101, 113, 121, 125, 128]
    assert bounds[-1] == NCHUNK

    # One DRAM tensor per stream group: each group is a dense contiguous
    # region (partition stride = the group's row length), giving the HBM
    # reads a compact footprint instead of 64 KiB-strided rows.  Group 0
    # carries the stage-1 weights prepended to its columns, so one DMA
    # delivers everything the first matmuls need.
    W8 = NCHUNK * 8
    xg = [
        nc.dram_tensor(
            f"x{g}",
            [128, (bounds[g + 1] - bounds[g]) * 256 + (W8 if g == 0 else 0)],
            F16, kind="ExternalInput").ap()
        for g in range(len(bounds) - 1)
    ]

    with tile.TileContext(nc) as tc, ExitStack() as ctx:
        wpool = ctx.enter_context(tc.tile_pool(name="wpool", bufs=1))
        # All x groups stay resident (8 MiB) so every stream DMA can be
        # issued up front; the sync HWDGE ring then drains back-to-back at
        # the HBM rate with no buffer-release gating.
        xpool = ctx.enter_context(tc.tile_pool(name="xpool", bufs=1))
        opool = ctx.enter_context(tc.tile_pool(name="opool", bufs=1))
        ppool = ctx.enter_context(tc.tile_pool(name="ppool", bufs=1, space="PSUM"))

        wn_sb = wpool.tile([8, N_OUT * MPD], F32, tag="wn")
        nc.scalar.dma_start(wn_sb[:], wn[:])

        # Stage 1: one fp16 matmul per 128-row chunk covers all 4 hi/lo
        # cross products; accumulate everything into one (8, 256) psum.
        psum1 = ppool.tile([8, 256], F32, tag="t")
        xts = []
        n_groups = len(bounds) - 1
        for g in range(n_groups):
            c0, c1 = bounds[g], bounds[g + 1]
            ncols = (c1 - c0) * 256 + (W8 if g == 0 else 0)
            xt = xpool.tile([128, ncols], F16, tag=f"x{g}")
            nc.sync.dma_start(xt[:], xg[g][:])
            xts.append(xt)
        w_sb = xts[0][:, 0:W8]
        for g in range(n_groups):
            c0, c1 = bounds[g], bounds[g + 1]
            xt = xts[g]
            off = W8 if g == 0 else 0
            for jj in range(c1 - c0):
                c = c0 + jj
                nc.tensor.matmul(
                    psum1[:],
                    lhsT=w_sb[:, c * 8:(c + 1) * 8],
                    rhs=xt[:, off + jj * 256:off + (jj + 1) * 256],
                    start=(c == 0),
                    stop=(c == NCHUNK - 1),
                )

        s8 = opool.tile([8, 256], F32, tag="s8")
        nc.vector.tensor_copy(s8[:], psum1[:])

        # Stage 2: contract over (k2, hw) against wn/32 (host-prescaled,
        # exact power-of-2), accumulating the two hx halves.
        psum2 = ppool.tile([128, 128], F32, tag="out")
        nc.tensor.matmul(psum2[:], lhsT=s8[:, 0:128], rhs=wn_sb[:],
                         start=True, stop=False)
        nc.tensor.matmul(psum2[:], lhsT=s8[:, 128:256], rhs=wn_sb[:],
                         start=False, stop=True)
        out_sb = opool.tile([128, 128], F32, tag="y")
        nc.vector.tensor_copy(out_sb[:], psum2[:])
        nc.sync.dma_start(y[:], out_sb[:])

    nc.compile()
    _CACHE["bounds"] = bounds
    return nc


def _split_f16(a: np.ndarray):
    hi = a.astype(np.float16)
    lo = (a - hi.astype(np.float32)).astype(np.float16)
    return hi, lo


def _prep_x(current_pose: np.ndarray) -> np.ndarray:
    """(256, 4096, 16) -> (8 cores, 128 partitions, NCHUNK*256 fp16 cols).

    Per core the stage-1 contraction matrix has row index (j*4 + k) and
    column (b*4 + r) with element pose[b, j, 4r+k].  Chunk Jc's 128x128
    tile lands in packed columns [Jc*256, Jc*256+128) as fp16 hi and
    [Jc*256+128, (Jc+1)*256) as fp16 lo.
    """
    a = current_pose.reshape(N_CORES, B_SH, N_IN, MPD, MPD)   # m b j r k
    t = a.transpose(0, 2, 4, 1, 3)                            # m j k b r
    c = t.reshape(N_CORES, NCHUNK, 128, 128)                  # m Jc p col
    c = np.ascontiguousarray(c.transpose(0, 2, 1, 3))         # m p Jc col
    hi, lo = _split_f16(c)
    packed = np.stack([hi, lo], axis=3)                       # m p Jc {hi,lo} col
    return np.ascontiguousarray(packed.reshape(N_CORES, 128, XCOLS))


def kernel(current_pose, w_current, w_next, h_out=1, w_out=1):
    global LAST_RESULT
    current_pose = np.asarray(current_pose, dtype=np.float32)
    w_current = np.asarray(w_current, dtype=np.float32)
    w_next = np.asarray(w_next, dtype=np.float32)

    if not TRACE:
        # bass_utils would honor a stray BASS_TRACE env var and then crash on
        # this image's missing NTFF hook module.
        os.environ.pop("BASS_TRACE", None)

    if "nc" not in _CACHE:
        _CACHE["nc"] = _build_program()
    nc = _CACHE["nc"]
    bounds = _CACHE["bounds"]

    xs = _prep_x(current_pose)

    # wc[j,k,c] flattened over rows (j,k); chunk Jc's (128, 4) block packed
    # into SBUF-image columns [Jc*8, Jc*8+4) as fp16 hi, [Jc*8+4, +8) as lo.
    wc_flat = w_current.reshape(JK, MPD)
    whi, wlo = _split_f16(wc_flat)
    w_img = np.concatenate(
        [whi.reshape(NCHUNK, 128, MPD), wlo.reshape(NCHUNK, 128, MPD)], axis=2)
    w_img = np.ascontiguousarray(
        w_img.transpose(1, 0, 2).reshape(128, NCHUNK * 8))

    # wn arranged (k2, (o,c)), pre-scaled by the exact 1/32 softmax constant
    # and replicated over the w-hi/lo axis for the stage-2 collapse.
    wn_t = (w_next.transpose(1, 0, 2).reshape(MPD, N_OUT * MPD)
            * np.float32(1.0 / N_OUT))
    wn8 = np.ascontiguousarray(np.concatenate([wn_t, wn_t], axis=0))

    in_maps = [
        {"wn": wn8,
         "x0": np.ascontiguousarray(np.concatenate(
             [w_img, xs[m][:, bounds[0] * 256:bounds[1] * 256]], axis=1)),
         **{f"x{g}": np.ascontiguousarray(
                xs[m][:, bounds[g] * 256:bounds[g + 1] * 256])
            for g in range(1, len(bounds) - 1)}}
        for m in range(N_CORES)
    ]
    res = run_bass_kernel_spmd(nc, in_maps, list(range(N_CORES)), trace=TRACE,
                               **TRACE_KWARGS)
    LAST_RESULT = res

    out = np.empty((B, 1, 1, N_OUT, POSE_DIM), dtype=np.float32)
    for m in range(N_CORES):
        ym = res.results[m]["y"]                      # (128=(b,r), 128=(o,c))
        out[m * B_SH:(m + 1) * B_SH, 0, 0] = (
            ym.reshape(B_SH, MPD, N_OUT, MPD)
            .transpose(0, 2, 1, 3).reshape(B_SH, N_OUT, POSE_DIM))
    return out


# revision 35
# speedup vs baseline: 1.0702x; 1.0191x over previous
"""Trainium2 Bass kernel for nn_BilinearSparseRouting (FC capsule routing layer).

Math (after constant-folding the softmax-over-a-constant, which is exactly 1/32):
    cp2[b,j]   = (pose[b,j] as 4x4) @ wc[j]            # (4,4) each
    S[b]       = (1/32) * sum_j cp2[b,j]               # (4,4)
    out[b,o]   = S[b] @ wn[o]                          # (4,4), o = 0..31
    output shape (256, 1, 1, 32, 16)

Device strategy (data-parallel over batch, 32 batches per core):
  Stage 1 is a 16384-term contraction per (b, r):
      T[(b,r), c] = sum_{(j,k)} pose[b, j, 4r+k] * wc[j, k, c]

  fp32 matmuls on the PE run as 2 half-speed passes (4 cyc/col); instead we
  split both operands into fp16 hi+lo pairs (x = x_hi + x_lo captures 22
  mantissa bits; fp16*fp16 products are exact in the fp32 PSUM accumulate).
  Packing rhs = [x_hi | x_lo] (256 cols) and lhsT = [w_hi | w_lo] (8 cols)
  computes all four cross products in ONE fp16 matmul per 128-row chunk:

      psum1[c + 4*hw, (b,r) + 128*hx] += w_hw_chunk.T @ x_hx_chunk

  at 1 cyc/col -- 4x faster than fp32 with ~fp32 accuracy (the lo*lo term
  is included, cutting the residual to ~2^-22).

  Stage 2 folds the quadrant collapse + the exact 1/32 scale into a single
  contraction against wn/32 replicated over the hw axis:

      out[(b,r),(o,c)] = sum_{k2,hw} s8[(k2,hw), (b,r)+128*hx] * wn8[(k2,hw),(o,c)]

  accumulated over hx in 2 tiny fp32 matmuls.

  The 8 MiB/core x stream is laid out on the host as per-group dense
  contiguous DRAM regions and streamed on the sync HWDGE ring with all
  destination tiles SBUF-resident, so the DMAs queue back-to-back at the
  practical HBM rate (~295 GB/s/core measured with all 8 cores streaming).
"""

import os
import sys

for _p in ("/opt/trn_rl_repo", "/root/.axon_site/_ro/trn_rl_repo"):
    if _p not in sys.path:
        sys.path.insert(0, _p)

# The kernel executes through the axon PJRT backend; a leftover cpu pin from a
# reference-running harness would hide the NeuronCores if jax has not
# initialized its backend yet.
os.environ.pop("JAX_PLATFORMS", None)

from contextlib import ExitStack  # noqa: E402

import numpy as np  # noqa: E402

import concourse.bacc as bacc  # noqa: E402
import concourse.mybir as mybir  # noqa: E402
import concourse.tile as tile  # noqa: E402
from concourse.bass_utils import run_bass_kernel_spmd  # noqa: E402

B = 256
N_IN = 4096
N_OUT = 32
MPD = 4
POSE_DIM = 16
N_CORES = 8
B_SH = B // N_CORES            # 32 batches per core
JK = N_IN * MPD                # 16384 contraction terms
NCHUNK = JK // 128             # 128 PE matmuls
XCOLS = NCHUNK * 256           # fp16 hi|lo packed columns of x

F32 = mybir.dt.float32
F16 = mybir.dt.float16

# Built once, reused across kernel() calls.
_CACHE = {}

# test.py hooks: set TRACE=True before calling kernel() to profile; the
# BassKernelResults of the last run lands in LAST_RESULT.
TRACE = False
TRACE_KWARGS = {}
LAST_RESULT = None


def _build_program():
    nc = bacc.Bacc("TRN2", target_bir_lowering=False, debug=False,
                   num_devices=N_CORES)
    wn = nc.dram_tensor("wn", [8, N_OUT * MPD], F32, kind="ExternalInput").ap()
    y = nc.dram_tensor("y", [128, 128], F32, kind="ExternalOutput").ap()

    # Group boundaries in chunks: small first group so the matmul stream
    # starts early, then a geometrically tapering tail.  A group's matmuls
    # can only start once its whole DMA lands (sem granularity), and the PE
    # consumes ~0.11 us/chunk vs ~0.18 us/chunk delivery -- so each trailing
    # group at <= ~1.4x the size of the next keeps the PE finishing a group
    # right as the next lands, cutting the post-stream PE trail from ~3 us
    # (one 30-chunk group) to ~0.5 us.
    bounds = [0, 4, 32, 60, 84, 100, 112, 119, 123, 126, 128]
    assert bounds[-1] == NCHUNK

    # One DRAM tensor per stream group: each group is a dense contiguous
    # region (partition stride = the group's row length), giving the HBM
    # reads a compact footprint instead of 64 KiB-strided rows.  Group 0
    # carries the stage-1 weights prepended to its columns, so one DMA
    # delivers everything the first matmuls need.
    W8 = NCHUNK * 8
    xg = [
        nc.dram_tensor(
            f"x{g}",
            [128, (bounds[g + 1] - bounds[g]) * 256 + (W8 if g == 0 else 0)],
            F16, kind="ExternalInput").ap()
        for g in range(len(bounds) - 1)
    ]

    with tile.TileContext(nc) as tc, ExitStack() as ctx:
        wpool = ctx.enter_context(tc.tile_pool(name="wpool", bufs=1))
        # All x groups stay resident (8 MiB) so every stream DMA can be
        # issued up front; the sync HWDGE ring then drains back-to-back at
        # the HBM rate with no buffer-release gating.
        xpool = ctx.enter_context(tc.tile_pool(name="xpool", bufs=1))
        opool = ctx.enter_context(tc.tile_pool(name="opool", bufs=1))
        ppool = ctx.enter_context(tc.tile_pool(name="ppool", bufs=1, space="PSUM"))

        wn_sb = wpool.tile([8, N_OUT * MPD], F32, tag="wn")
        nc.scalar.dma_start(wn_sb[:], wn[:])

        # Stage 1: one fp16 matmul per 128-row chunk covers all 4 hi/lo
        # cross products; accumulate everything into one (8, 256) psum.
        psum1 = ppool.tile([8, 256], F32, tag="t")
        xts = []
        n_groups = len(bounds) - 1
        for g in range(n_groups):
            c0, c1 = bounds[g], bounds[g + 1]
            ncols = (c1 - c0) * 256 + (W8 if g == 0 else 0)
            xt = xpool.tile([128, ncols], F16, tag=f"x{g}")
            nc.sync.dma_start(xt[:], xg[g][:])
            xts.append(xt)
        w_sb = xts[0][:, 0:W8]
        for g in range(n_groups):
            c0, c1 = bounds[g], bounds[g + 1]
            xt = xts[g]
            off = W8 if g == 0 else 0
            for jj in range(c1 - c0):
                c = c0 + jj
                nc.tensor.matmul(
                    psum1[:],
                    lhsT=w_sb[:, c * 8:(c + 1) * 8],
                    rhs=xt[:, off + jj * 256:off + (jj + 1) * 256],
                    start=(c == 0),
                    stop=(c == NCHUNK - 1),
                )

        s8 = opool.tile([8, 256], F32, tag="s8")
        nc.vector.tensor_copy(s8[:], psum1[:])

        # Stage 2: contract over (k2, hw) against wn/32 (host-prescaled,
        # exact power-of-2), accumulating the two hx halves.
        psum2 = ppool.tile([128, 128], F32, tag="out")
        nc.tensor.matmul(psum2[:], lhsT=s8[:, 0:128], rhs=wn_sb[:],
                         start=True, stop=False)
        nc.tensor.matmul(psum2[:], lhsT=s8[:, 128:256], rhs=wn_sb[:],
                         start=False, stop=True)
        out_sb = opool.tile([128, 128], F32, tag="y")
        nc.vector.tensor_copy(out_sb[:], psum2[:])
        nc.sync.dma_start(y[:], out_sb[:])

    nc.compile()
    _CACHE["bounds"] = bounds
    return nc


def _split_f16(a: np.ndarray):
    hi = a.astype(np.float16)
    lo = (a - hi.astype(np.float32)).astype(np.float16)
    return hi, lo


def _prep_x(current_pose: np.ndarray) -> np.ndarray:
    """(256, 4096, 16) -> (8 cores, 128 partitions, NCHUNK*256 fp16 cols).

    Per core the stage-1 contraction matrix has row index (j*4 + k) and
    column (b*4 + r) with element pose[b, j, 4r+k].  Chunk Jc's 128x128
    tile lands in packed columns [Jc*256, Jc*256+128) as fp16 hi and
    [Jc*256+128, (Jc+1)*256) as fp16 lo.
    """
    a = current_pose.reshape(N_CORES, B_SH, N_IN, MPD, MPD)   # m b j r k
    t = a.transpose(0, 2, 4, 1, 3)                            # m j k b r
    c = t.reshape(N_CORES, NCHUNK, 128, 128)                  # m Jc p col
    c = np.ascontiguousarray(c.transpose(0, 2, 1, 3))         # m p Jc col
    hi, lo = _split_f16(c)
    packed = np.stack([hi, lo], axis=3)                       # m p Jc {hi,lo} col
    return np.ascontiguousarray(packed.reshape(N_CORES, 128, XCOLS))


def kernel(current_pose, w_current, w_next, h_out=1, w_out=1):
    global LAST_RESULT
    current_pose = np.asarray(current_pose, dtype=np.float32)
    w_current = np.asarray(w_current, dtype=np.float32)
    w_next = np.asarray(w_next, dtype=np.float32)

    if not TRACE:
        # bass_utils would honor a stray BASS_TRACE env var and then crash on
        # this image's missing NTFF hook module.
        os.environ.pop("BASS_TRACE", None)

    if "nc" not in _CACHE:
        _CACHE["nc"] = _build_program()
    nc = _CACHE["nc"]
    bounds = _CACHE["bounds"]

    xs = _prep_x(current_pose)

    # wc[j,k,c] flattened over rows (j,k); chunk Jc's (128, 4) block packed
    # into SBUF-image columns [Jc*8, Jc*8+4) as fp16 hi, [Jc*8+4, +8) as lo.
    wc_flat = w_current.reshape(JK, MPD)
    whi, wlo = _split_f16(wc_flat)
    w_img = np.concatenate(
        [whi.reshape(NCHUNK, 128, MPD), wlo.reshape(NCHUNK, 128, MPD)], axis=2)
    w_img = np.ascontiguousarray(
        w_img.transpose(1, 0, 2).reshape(128, NCHUNK * 8))

    # wn arranged (k2, (o,c)), pre-scaled by the exact 1/32 softmax constant
    # and replicated over the w-hi/lo axis for the stage-2 collapse.
    wn_t = (w_next.transpose(1, 0, 2).reshape(MPD, N_OUT * MPD)
            * np.float32(1.0 / N_OUT))
    wn8 = np.ascontiguousarray(np.concatenate([wn_t, wn_t], axis=0))

    in_maps = [
        {"wn": wn8,
         "x0": np.ascontiguousarray(np.concatenate(
             [w_img, xs[m][:, bounds[0] * 256:bounds[1] * 256]], axis=1)),
         **{f"x{g}": np.ascontiguousarray(
                xs[m][:, bounds[g] * 256:bounds[g + 1] * 256])
            for g in range(1, len(bounds) - 1)}}
        for m in range(N_CORES)
    ]
    res = run_bass_kernel_spmd(nc, in_maps, list(range(N_CORES)), trace=TRACE,
                               **TRACE_KWARGS)
    LAST_RESULT = res

    out = np.empty((B, 1, 1, N_OUT, POSE_DIM), dtype=np.float32)
    for m in range(N_CORES):
        ym = res.results[m]["y"]                      # (128=(b,r), 128=(o,c))
        out[m * B_SH:(m + 1) * B_SH, 0, 0] = (
            ym.reshape(B_SH, MPD, N_OUT, MPD)
            .transpose(0, 2, 1, 3).reshape(B_SH, N_OUT, POSE_DIM))
    return out
